# revision 17
# baseline (speedup 1.0000x reference)
"""Trainium2 Bass kernel for a DP-GAT layer (dense masked attention).

Computes, for x:[B,N,D], A_shape:[N,N] (0/1 adjacency), q,k,v:[D,D]:
    Q = x@q ; K = x@k
    S = Q @ K^T / sqrt(D)
    W = exp(8*tanh(S/8)) * A_shape
    out = (W / W.sum(-1, keepdims=True)) @ x @ v

Sharding: rows of N split across 8 NeuronCores (1024 rows each), SPMD,
no collectives. Each core streams its row-block of the mask, computes
scores in a flash-attention-style fused loop, and writes its row-block
of the output. Host scatters inputs / gathers outputs.

The kernel bottleneck is the two ScalarE table lookups (tanh, exp) over
every score element. To relieve ScalarE, half the key-tile groups
(PAIR_G) replace the exp with a Schraudolph bit-hack pair on the DVE:
  i = int16(round(u*SCH_A + B))  (tensor_scalar, 4x mode)
  w ~= bitcast_fp16(i)           == e^(8u) * (1 + sawtooth(~3%))
Two half-period-shifted estimates are summed to cut the sawtooth to
~1.5%; their uniform gain (~2.034) is matched on the exact-exp path via
a bias (exp(8u + ln 2.034)) so rows mixing both paths normalize
consistently. End-to-end output error ~3e-3 (validated vs numpy).

Device-side flow (per core, per batch):
    KT  = k^T @ x^T  (fp16)          [D, N]
    QT  = q^T @ xrows^T (fp16)       [D, RB]
    xr  = x rows + ones column       [N, D+1] fp16 (pure DMA, no compute)
    per i-chunk of 512 query rows:
      per group of 4 key-tiles (512 keys):
        S^T  = KT_tile^T @ QT_chunk      -> PSUM [128, 4, 512] fp32
        u    = tanh(S^T / (8*sqrt(D)))   -> SBUF fp16  (ScalarE)
        w    = exp path (ScalarE) or Schraudolph pair (DVE)
        [1 group later]  p = w * maskT   (DVE; lag keeps DVE off the
                                          ScalarE exp's latency)
        [2 groups later] acc += p^T @ xr -> PSUM  (col 128 = rowsum)
      per 128-row slot: px = acc * (1/rowsum) -> fp16; transpose px on
      the DMA xbar; out = px^T-matmul v -> DMA to DRAM. The @v lands
      after normalization so no x@v prep (and its PSUM->SBUF copies on
      the DVE) is needed.

PSUM: score 4 banks + acc 2 + prep/vout 2. The first PV matmul into
each acc bank uses start=True (clears the whole bank), so no dummy
zeroing matmuls are needed.
"""

import math
import sys
from contextlib import ExitStack

import numpy as np

try:
    import concourse.bass as bass  # noqa: F401
except ImportError:  # pragma: no cover
    sys.path.insert(0, "/opt/trn_rl_repo")
    import concourse.bass as bass  # noqa: F401

import concourse.mybir as mybir
import concourse.tile as tile
from concourse import bacc
from concourse.bass_utils import run_bass_kernel_spmd

F32 = mybir.dt.float32
F16 = mybir.dt.float16
I16 = mybir.dt.int16

B, N, D = 4, 8192, 128
NCORES = 8
RB = N // NCORES  # query rows per core

IC = 512          # query-row chunk (free dim of score matmuls)
NIC = RB // IC    # i-chunks per core
JG = 4            # key 128-tiles per score group
NJT = N // 128    # key tiles total
NG = NJT // JG    # groups per i-chunk
CH = JG * 128     # xt prep chunk width (chunk g produces what group g consumes)

SCH_A = 1024.0 * 8.0 * 1.4426950408889634
SCH_B1 = 15360.0 + 200.0
SCH_B2 = 15360.0 - 312.0
# pair-path groups (Schraudolph on DVE), interleaved with exact-exp groups
PAIR_G = frozenset({0, 2, 4, 6, 8, 10, 12, 14})
PV_LAG = 2        # groups between weight production and PV consumption
CARRY_ON = False  # hoist next i-chunk's first weight stage across the drain


def build_program():
    nc = bacc.Bacc("TRN2", target_bir_lowering=False, debug=False)

    xt = nc.dram_tensor("xt", [B, D, N], F16, kind="ExternalInput").ap()
    xqt = nc.dram_tensor("xqt", [B, D, RB], F16, kind="ExternalInput").ap()
    xpad = nc.dram_tensor("xpad", [B, N, 130], F16, kind="ExternalInput").ap()
    maskT = nc.dram_tensor("maskT", [N, RB], F16, kind="ExternalInput").ap()
    q_d = nc.dram_tensor("q16", [D, D], F16, kind="ExternalInput").ap()
    k_d = nc.dram_tensor("k16", [D, D], F16, kind="ExternalInput").ap()
    v_d = nc.dram_tensor("v", [D, D], F16, kind="ExternalInput").ap()
    out_d = nc.dram_tensor("out", [B, RB, D], F32, kind="ExternalOutput").ap()

    # [128, key-tile, query-col] view of the transposed mask block
    maskT_r = maskT.rearrange("(t p) i -> p t i", p=128)

    tanh_scale = 1.0 / (8.0 * math.sqrt(float(D)))

    with tile.TileContext(nc) as tc, ExitStack() as ctx:
        consts = ctx.enter_context(tc.tile_pool(name="consts", bufs=1))
        kt_pool = ctx.enter_context(tc.tile_pool(name="kt", bufs=2))
        qt_pool = ctx.enter_context(tc.tile_pool(name="qt", bufs=2))
        xr_pool = ctx.enter_context(tc.tile_pool(name="xr", bufs=2))
        xc_pool = ctx.enter_context(tc.tile_pool(name="xc", bufs=3))
        m_pool = ctx.enter_context(tc.tile_pool(name="m", bufs=4))
        u_pool = ctx.enter_context(tc.tile_pool(name="u", bufs=2))
        w_pool = ctx.enter_context(tc.tile_pool(name="w", bufs=3))
        i1_pool = ctx.enter_context(tc.tile_pool(name="i1", bufs=2))
        i2_pool = ctx.enter_context(tc.tile_pool(name="i2", bufs=2))
        p_pool = ctx.enter_context(tc.tile_pool(name="p", bufs=4))
        px_pool = ctx.enter_context(tc.tile_pool(name="px", bufs=4))
        pxt_pool = ctx.enter_context(tc.tile_pool(name="pxt", bufs=4))
        ob_pool = ctx.enter_context(tc.tile_pool(name="ob", bufs=4))
        rs_pool = ctx.enter_context(tc.tile_pool(name="rs", bufs=4))
        prep_ps = ctx.enter_context(tc.tile_pool(name="prep_ps", bufs=1, space="PSUM"))
        st_ps = ctx.enter_context(tc.tile_pool(name="st_ps", bufs=1, space="PSUM"))
        acc_ps = ctx.enter_context(tc.tile_pool(name="acc_ps", bufs=1, space="PSUM"))

        gain_bias = consts.tile([128, 1], F32)
        nc.vector.memset(gain_bias[:], 0.7102396)
        q_sb = consts.tile([D, D], F16)
        nc.sync.dma_start(q_sb[:], q_d[:])
        k_sb = consts.tile([D, D], F16)
        nc.sync.dma_start(k_sb[:], k_d[:])
        v_sb = consts.tile([D, D], F16)
        nc.sync.dma_start(v_sb[:], v_d[:])

        tiles = {}  # b -> (kt, qt, xr)

        def prep_head(b):
            """Allocate batch-b tiles; compute QT."""
            kt = kt_pool.tile([128, N], F16)
            qt = qt_pool.tile([128, RB], F16)
            xr = xr_pool.tile([128, NJT, 130], F16)
            tiles[b] = (kt, qt, xr)
            xq = qt_pool.tile([128, RB], F16, tag="xq")
            nc.sync.dma_start(xq[:], xqt[b])
            qch = min(CH, RB)
            for c in range(RB // qch):
                pq = prep_ps.tile([128, qch], F32, tag="prep")
                nc.tensor.matmul(
                    pq[:], q_sb[:], xq[:, c * qch : (c + 1) * qch],
                    start=True, stop=True,
                )
                nc.vector.tensor_copy(qt[:, c * qch : (c + 1) * qch], pq[:])

        def prep_chunk(b, c):
            """kt columns (PE+DVE) and xr tiles (pure DMA) for chunk c."""
            kt, _, xr = tiles[b]
            xp_r = xpad[b].rearrange("(t p) d -> p t d", p=128)
            nc.sync.dma_start(
                xr[:, c * JG : (c + 1) * JG, :], xp_r[:, c * JG : (c + 1) * JG, :]
            )
            xc = xc_pool.tile([128, CH], F16)
            nc.sync.dma_start(xc[:], xt[b][:, c * CH : (c + 1) * CH])
            pk = prep_ps.tile([128, CH], F32, tag="prep")
            nc.tensor.matmul(pk[:], k_sb[:], xc[:], start=True, stop=True)
            nc.vector.tensor_copy(kt[:, c * CH : (c + 1) * CH], pk[:])

        def weights_w(b, ic, g):
            """Scores + tanh + w for group g; returns (w, m) pre-mask."""
            kt, qt, _ = tiles[b]
            stp = st_ps.tile([128, JG, IC], F32)
            for j in range(JG):
                nc.tensor.matmul(
                    stp[:, j],
                    kt[:, (g * JG + j) * 128 : (g * JG + j + 1) * 128],
                    qt[:, ic * IC : (ic + 1) * IC],
                    start=True, stop=True,
                )
            u = u_pool.tile([128, JG, IC], F16)
            nc.scalar.activation(
                u[:], stp[:], mybir.ActivationFunctionType.Tanh, scale=tanh_scale
            )
            w = w_pool.tile([128, JG, IC], F16)
            if g in PAIR_G:
                # Schraudolph pair entirely on the DVE; ScalarE only tanh
                i1 = i1_pool.tile([128, JG, IC], I16)
                nc.vector.tensor_scalar(
                    i1[:], u[:], SCH_A, SCH_B1,
                    mybir.AluOpType.mult, mybir.AluOpType.add,
                )
                i2 = i2_pool.tile([128, JG, IC], I16)
                nc.vector.tensor_scalar(
                    i2[:], u[:], SCH_A, SCH_B2,
                    mybir.AluOpType.mult, mybir.AluOpType.add,
                )
                nc.vector.tensor_add(w[:], i1[:].bitcast(F16), i2[:].bitcast(F16))
            else:
                # bias = ln(2.03448): match the pair path's uniform gain
                nc.scalar.activation(
                    w[:], u[:], mybir.ActivationFunctionType.Exp,
                    scale=8.0, bias=gain_bias[:],
                )
            m = m_pool.tile([128, JG, IC], F16)
            nc.sync.dma_start(
                m[:], maskT_r[:, g * JG : (g + 1) * JG, ic * IC : (ic + 1) * IC]
            )
            return (w, m)

        def emit_mul(wm):
            """Deferred mask-mul: by the time it runs, w is long since done."""
            w, m = wm
            p = p_pool.tile([128, JG, IC], F16)
            nc.vector.tensor_mul(p[:], w[:], m[:])
            return p

        def emit_pv(b, g, p, acc):
            _, _, xr = tiles[b]
            for j in range(JG):
                for s in range(IC // 128):
                    # start=True on the first matmul touching each acc bank
                    # (s 0/1 -> bank 0, s 2/3 -> bank 1) clears the bank
                    nc.tensor.matmul(
                        acc[:, s * 256 : s * 256 + 129],
                        p[:, j, s * 128 : (s + 1) * 128],
                        xr[:, g * JG + j, 0:129],
                        start=(g == 0 and j == 0 and s % 2 == 0),
                        stop=(g == NG - 1 and j == JG - 1),
                        skip_group_check=True,
                    )

        def normalize(b, ic, acc):
            """acc -> P-hat x (fp16) -> xbar transpose -> @v -> DRAM."""
            for s in range(IC // 128):
                rs = rs_pool.tile([128, 1], F32)
                nc.vector.reciprocal(rs[:], acc[:, s * 256 + 128 : s * 256 + 129])
                px = px_pool.tile([128, 128], F16)
                nc.vector.tensor_scalar_mul(
                    px[:], acc[:, s * 256 : s * 256 + 128], rs[:]
                )
                pxt = pxt_pool.tile([128, 128], F16)
                nc.sync.dma_start_transpose(pxt[:], px[:])
                vout = prep_ps.tile([128, 128], F32, tag="vout")
                nc.tensor.matmul(vout[:], pxt[:], v_sb[:], start=True, stop=True)
                ob = ob_pool.tile([128, 128], F32)
                nc.vector.tensor_copy(ob[:], vout[:])
                nc.sync.dma_start(
                    out_d[b, ic * IC + s * 128 : ic * IC + (s + 1) * 128, :],
                    ob[:],
                )

        # Flat software pipeline over (b, ic, g): the mask-mul runs one group
        # behind the weight stage (so the in-order DVE never blocks on the
        # ScalarE exp), and PV runs PV_LAG groups behind.
        prep_head(0)
        carry = None  # weight stage hoisted from the previous ic boundary
        for b in range(B):
            for ic in range(NIC):
                if ic == NIC - 1 and b + 1 < B:
                    prep_head(b + 1)
                acc = acc_ps.tile([128, 1024], F32)
                if b == 0 and ic == 0:
                    prep_chunk(0, 0)
                    prep_chunk(0, 1)
                if carry is None:
                    wq = []    # groups with w emitted, mul pending
                    pq = []    # groups with p emitted, PV pending
                    carried = False
                else:
                    wq = [carry]
                    pq = []
                    carried = True
                    carry = None
                for g in range(NG):
                    if b == 0 and ic == 0 and g + 2 < NG:
                        prep_chunk(0, g + 2)
                    if ic == NIC - 1 and b + 1 < B:
                        prep_chunk(b + 1, g)
                    if g == 0 and carried:
                        continue  # weight stage was hoisted across the drain
                    wq.append((g, weights_w(b, ic, g)))
                    if len(wq) > 1:
                        mg, wm = wq.pop(0)
                        pq.append((mg, emit_mul(wm)))
                    if len(pq) >= PV_LAG:
                        pg, pp = pq.pop(0)
                        emit_pv(b, pg, pp, acc)
                # hoist next chunk's group-0 weight stage before the drain
                nb, nic = (b, ic + 1) if ic + 1 < NIC else (b + 1, 0)
                if nb < B and CARRY_ON:
                    carry = (0, weights_w(nb, nic, 0))
                for mg, wm in wq:
                    pq.append((mg, emit_mul(wm)))
                for pg, pp in pq:
                    emit_pv(b, pg, pp, acc)
                normalize(b, ic, acc)

    nc.compile()
    return nc


_CACHED_NC = None


def _get_program():
    global _CACHED_NC
    if _CACHED_NC is None:
        _CACHED_NC = build_program()
    return _CACHED_NC


def make_in_maps(x, A_shape, q, k, v):
    x = np.ascontiguousarray(x, dtype=np.float32)
    xt = np.ascontiguousarray(x.transpose(0, 2, 1)).astype(np.float16)  # [B, D, N]
    x16 = x.astype(np.float16)
    xpad = np.zeros((B, N, 130), dtype=np.float16)
    xpad[:, :, 0:D] = x16
    xpad[:, :, D] = 1.0
    q16 = np.ascontiguousarray(q, dtype=np.float32).astype(np.float16)
    k16 = np.ascontiguousarray(k, dtype=np.float32).astype(np.float16)
    v16 = np.ascontiguousarray(v, dtype=np.float32).astype(np.float16)
    in_maps = []
    for c in range(NCORES):
        r0 = c * RB
        xqt = np.ascontiguousarray(
            x[:, r0 : r0 + RB, :].transpose(0, 2, 1)
        ).astype(np.float16)
        maskT = np.ascontiguousarray(A_shape[r0 : r0 + RB, :].T, dtype=np.float16)
        in_maps.append(
            {
                "xt": xt,
                "xqt": xqt,
                "xpad": xpad,
                "maskT": maskT,
                "q16": q16,
                "k16": k16,
                "v": v16,
            }
        )
    return in_maps


def kernel(x, A_shape, q, k, v):
    nc = _get_program()
    in_maps = make_in_maps(x, A_shape, q, k, v)
    res = run_bass_kernel_spmd(nc, in_maps, list(range(NCORES)))
    out = np.concatenate([res.results[c]["out"] for c in range(NCORES)], axis=1)
    return out.astype(np.float32)
